# revision 30
# baseline (speedup 1.0000x reference)
"""MQA attention (16 Q heads, 1 KV head) on 8 trn2 NeuronCores.

Sharding: data-parallel on batch (2) x tensor-parallel on Q heads (4 per
core). Each core computes K/V for its batch (replicated within the batch
group), attention for its 4 heads, and a row-parallel o_proj partial; the
host sums the 4 partials per batch.

v3 kernel layout (all matmul contractions on partitions):
  xT [1024, 2048] (host pre-transposed); x/wq/wkk DMAs are kc-chunked
  across the 3 HWDGE queues so projection matmuls start ~4us in
  qT = wqT.T @ xT -> [256, 2048] as 2 head-pair tiles [128, 2048]
  kT duplicated to both partition halves -> row-packed score matmuls
  scoresT [k, q] per (j, pair, kb) in a shared PSUM ring [128, 1024]
  (the same ring carries proj and o_proj partials: 4 banks + 4 att banks)
  exp on ScalarE PSUM->SBUF bf16 with per-partition (=per-key) mask bias
  minus ln32 (cancels in softmax)
  PV: lhsT = vaug [128, 65] ([v | ones]) bf16, rhs = ex [128, 512] bf16,
  4-way interleaved accumulation (A/B heads x even/odd kb) to keep
  same-group matmuls apart; even+odd summed on DVE into the tmp tile
  normalization: reciprocal + DMA partition-broadcast + DVE multiply
  o_proj: out[q, hidden] partial = attnT.T @ woT, K=256, interleaved per-j

Schedule: scores/exp for (j0,p0) wavefront against the kc-paced DMAs and
the per-j projections (proj pieces scattered between units); all other
(j,pair) blocks run pair-major so each PV block gets a long reuse window.
ScalarE exp is the hard floor (~131us: 16.8M exps at 1 elem/cycle/lane);
the schedule keeps PE (the other ~150us floor) from ever blocking on it.
"""
import math
import sys

sys.path.insert(0, "/opt/trn_rl_repo")

import ml_dtypes
import numpy as np

import concourse.bass as bass
import concourse.bacc as bacc
import concourse.tile as tile
from concourse import mybir
from concourse.bass_utils import run_bass_kernel_spmd
from concourse.tile_rust import add_dep_helper

HIDDEN = 1024
NH = 16
D = 64
B = 2
S = 2048
NCORES = 8
HEADS_PER_CORE = 4
KB = S // 128   # 16 key blocks
QC = S // 512   # 4 query chunks
P = 128
LNSH = math.log(32.0)  # softmax shift; cancels in normalization

F32 = mybir.dt.float32
F32R = mybir.dt.float32r
BF16 = mybir.dt.bfloat16
EXPF = mybir.ActivationFunctionType.Exp
MULT = mybir.AluOpType.mult
ADD = mybir.AluOpType.add

_CACHE = {}


def build_kernel(num_devices=NCORES):
    nc = bacc.Bacc("TRN2", target_bir_lowering=False, debug=False,
                   num_devices=num_devices)

    xT = nc.dram_tensor("xT", [P, QC, 8, 512], BF16, kind="ExternalInput")
    wqT = nc.dram_tensor("wqT", [HIDDEN, 256], BF16, kind="ExternalInput")
    wkkT = nc.dram_tensor("wkkT", [HIDDEN, 128], BF16, kind="ExternalInput")
    wvT = nc.dram_tensor("wvT", [HIDDEN, D], BF16, kind="ExternalInput")
    identT = nc.dram_tensor("identT", [D, D], F32R, kind="ExternalInput")
    woT = nc.dram_tensor("woT", [256, HIDDEN], BF16, kind="ExternalInput")
    bias2d = nc.dram_tensor("bias2d", [P, KB], F32, kind="ExternalInput")
    ones2d = nc.dram_tensor("ones2d", [P, KB], BF16, kind="ExternalInput")
    out = nc.dram_tensor("out", [S, HIDDEN], BF16, kind="ExternalOutput")
    # internal DRAM bounce for the per-query 1/denom row broadcast
    bounce = nc.dram_tensor("bounce", [QC, 2, 2, 512], F32)

    queues = [nc.sync, nc.scalar, nc.gpsimd]

    with tile.TileContext(nc) as tc:
        with tc.tile_pool(name="persist", bufs=1) as persist:
            xts = [persist.tile([P, 8, 512], BF16, name=f"xt{jj}")
                   for jj in range(QC)]
            qt = persist.tile([P, 2, S], BF16)          # qT head pairs
            kt = persist.tile([P, S], BF16)             # kT dup both halves
            vt = persist.tile([D, S], F32R)
            vaug = persist.tile([P, KB, D + 1], BF16)   # [v | ones]
            attnT_js = [persist.tile([P, 2, 512], BF16, name=f"attnT{jj}")
                        for jj in range(QC)]
            wq_sb = persist.tile([P, 8, 256], BF16)
            wkk_sb = persist.tile([P, 8, 128], BF16)
            wv_sb = persist.tile([P, 8, D], BF16)
            id_sb = persist.tile([D, D], F32R)
            wo_sb = persist.tile([P, 2, HIDDEN], BF16)
            bias_sb = persist.tile([P, KB], F32)

            # ---- input DMAs ----
            # kc-chunked so k0/q0 matmuls can start as soon as the first
            # chunks land; queue kc%3 carries [wkk_kc, wq_kc, x0_kc].
            nc.sync.dma_start(out=bias_sb, in_=bias2d[:, :])
            warm = persist.tile([P, 1], F32)
            nc.scalar.activation(warm, bias_sb[:, 0:1], EXPF)

            wkk_ap = wkkT.ap().rearrange("(kc p) m -> p kc m", p=P)
            wq_ap = wqT.ap().rearrange("(kc p) m -> p kc m", p=P)
            nc.sync.dma_start(out=wkk_sb[:, 0:4, :], in_=wkk_ap[:, 0:4, :])
            nc.scalar.dma_start(out=wkk_sb[:, 4:8, :],
                                in_=wkk_ap[:, 4:8, :])
            nc.gpsimd.dma_start(out=wq_sb[:, 0:4, :], in_=wq_ap[:, 0:4, :])
            nc.scalar.dma_start(out=wq_sb[:, 4:8, :], in_=wq_ap[:, 4:8, :])
            for i, q in ((0, nc.sync), (1, nc.gpsimd),
                         (2, nc.scalar), (3, nc.sync)):
                q.dma_start(out=xts[0][:, 2 * i:2 * i + 2, :],
                            in_=xT[:, 0, 2 * i:2 * i + 2, :])
            nc.gpsimd.dma_start(
                out=wv_sb, in_=wvT.ap().rearrange("(kc p) m -> p kc m", p=P))
            nc.scalar.dma_start(out=id_sb, in_=identT[:, :])
            nc.scalar.dma_start(out=vaug[:, :, D:D + 1], in_=ones2d[:, :])
            for jj in range(1, QC):
                nc.sync.dma_start(out=xts[jj][:, 0:4, :],
                                  in_=xT[:, jj, 0:4, :])
                nc.scalar.dma_start(out=xts[jj][:, 4:8, :],
                                    in_=xT[:, jj, 4:8, :])
            for t in range(2):
                nc.gpsimd.dma_start(out=wo_sb[:, t, :],
                                    in_=woT[t * P:(t + 1) * P, :])

            # ---- PE warm-up burst: ~24 junk matmuls unthrottle the HAM
            # clock gate (4096-cycle activity window) before proj0 ----
            wjunk = persist.tile([P, 512], BF16)
            nc.vector.memset(wjunk, 0.0)
            with tc.tile_pool(name="warm_ps", bufs=1,
                              space="PSUM") as wps:
                wslot = wps.tile([P, 512], F32, tag="w")
                for _ in range(24):
                    nc.tensor.matmul(wslot, lhsT=wjunk[:, 0:128],
                                     rhs=wjunk, start=True, stop=True)

            with tc.tile_pool(name="ps", bufs=2, space="PSUM") as psp, \
                 tc.tile_pool(name="att_ps", bufs=1, space="PSUM") as attp, \
                 tc.tile_pool(name="ex_sb", bufs=24) as expp, \
                 tc.tile_pool(name="norm_sb", bufs=3) as normp, \
                 tc.tile_pool(name="o_sb", bufs=3) as osb:

                ex_map = {}
                att_map = {}
                pv_pending = {}   # pair ord -> [(jj, pair, kb), ...]
                normed = {jj: 0 for jj in range(QC)}
                norm_cnt = [0]
                out_qi = [0]
                oproj_jobs = []   # (jj, chunk) emitted spread-out, delayed
                odelay = [0]

                def emit_kv(jj, qfirst=False):
                    slot = psp.tile([P, 1024], F32, tag="sc", name=f"kv{jj}")
                    for kc in range(8):
                        nc.tensor.matmul(
                            slot[:, 0:512], lhsT=wkk_sb[:, kc, :],
                            rhs=xts[jj][:, kc, :],
                            start=(kc == 0), stop=(kc == 7))
                    nc.vector.tensor_copy(
                        kt[:, jj * 512:(jj + 1) * 512], slot[:, 0:512])
                    if qfirst:
                        emit_qproj(jj)
                    for kc in range(8):
                        nc.tensor.matmul(
                            slot[0:D, 512:1024], lhsT=wv_sb[:, kc, :],
                            rhs=xts[jj][:, kc, :],
                            start=(kc == 0), stop=(kc == 7))
                    nc.vector.tensor_copy(
                        vt[:, jj * 512:(jj + 1) * 512], slot[0:D, 512:1024])
                    if qfirst:
                        emit_vT(jj)

                def emit_vT(jj):
                    tslot = psp.tile([P, 1024], F32, tag="sc", name=f"vT{jj}")
                    tslot_r = tslot.bitcast(F32R)
                    for c in range(4):
                        blk = 4 * jj + c
                        nc.tensor.transpose(
                            tslot_r[:, c * D:(c + 1) * D],
                            vt[0:D, blk * P:(blk + 1) * P],
                            id_sb[:, :])
                    nc.vector.tensor_copy(
                        vaug[:, 4 * jj:4 * jj + 4, 0:D],
                        tslot_r[:, 0:4 * D].rearrange("p (c d) -> p c d", c=4))

                def emit_qproj(jj):
                    qslot = psp.tile([P, 1024], F32, tag="sc", name=f"q{jj}")
                    for pair in range(2):
                        for kc in range(8):
                            nc.tensor.matmul(
                                qslot[:, pair * 512:(pair + 1) * 512],
                                lhsT=wq_sb[:, kc, pair * P:(pair + 1) * P],
                                rhs=xts[jj][:, kc, :],
                                start=(kc == 0), stop=(kc == 7))
                    nc.vector.tensor_copy(
                        qt[:, :, jj * 512:(jj + 1) * 512],
                        qslot.rearrange("p (pr m) -> p pr m", pr=2))

                def emit_norm(jj, pair, tiles):
                    attAB_e, attAB_o = tiles
                    last = jj == QC - 1
                    dengs = {1: nc.sync if last else nc.gpsimd,
                             0: nc.gpsimd}
                    for h01 in (1, 0):  # head B first (its DMA is the tail)
                        hs = h01 * 512
                        deng = dengs[h01]
                        tmpe = normp.tile([D + 1, 512], F32, tag="tmpe")
                        nc.vector.tensor_copy(
                            tmpe, attAB_e[:, hs:hs + 512])
                        tmp = normp.tile([D + 1, 512], F32, tag="tmp")
                        nc.vector.scalar_tensor_tensor(
                            out=tmp, in0=attAB_o[:, hs:hs + 512],
                            scalar=1.0, in1=tmpe, op0=MULT, op1=ADD)
                        ds = normp.tile([D, 8], F32, tag="ds")
                        deng.dma_start(out=ds, in_=tmp[D:D + 1, :])
                        rs = normp.tile([D, 8], F32, tag="rs")
                        nc.vector.reciprocal(out=rs, in_=ds)
                        bc = normp.tile([D, 1, 512], F32, tag="bc")
                        wdma = deng.dma_start(
                            out=bounce[jj, pair, h01, :], in_=rs)
                        rdma = deng.dma_start(
                            out=bc,
                            in_=bounce[jj, pair,
                                       h01, :].partition_broadcast(D))
                        add_dep_helper(rdma.ins, wdma.ins,
                                       reason="bounce RAW")
                        if h01 == 0:
                            nc.vector.tensor_mul(
                                attnT_js[jj][0:D, pair, :],
                                tmp[0:D, :], bc[:, 0, :])
                        else:
                            nt = normp.tile([D, 512], BF16, tag="nt")
                            nc.vector.tensor_mul(nt, tmp[0:D, :],
                                                 bc[:, 0, :])
                            deng.dma_start(
                                out=attnT_js[jj][D:P, pair, :], in_=nt)
                    norm_cnt[0] += 1
                    normed[jj] += 1
                    if normed[jj] == 2:
                        oproj_jobs.extend((jj, c) for c in range(4))
                        odelay[0] = 10

                def emit_pv(jj, pair, kb):
                    if kb == 0:
                        tiles = tuple(
                            attp.tile([D + 1, 1024], F32, tag=tg,
                                      name=f"{tg}_{jj}_{pair}")
                            for tg in ("attE", "attO"))
                        att_map[(jj, pair)] = tiles
                    tiles = att_map[(jj, pair)]
                    eo = kb % 2
                    attP = tiles[eo]
                    ex = ex_map.pop((jj, pair, kb))
                    nc.tensor.matmul(
                        attP[:, 0:512], lhsT=vaug[:, kb, :],
                        rhs=ex[:, 0:512],
                        start=(kb == eo), stop=(kb == KB - 2 + eo))
                    nc.tensor.matmul(
                        attP[:, 512:1024], lhsT=vaug[:, kb, :],
                        rhs=ex[:, 512:1024],
                        start=(kb == eo), stop=(kb == KB - 2 + eo))
                    if kb == KB - 1:
                        emit_norm(jj, pair, tiles)
                        del att_map[(jj, pair)]

                def drain_pv(limit):
                    # pair-order gating: a pair's PVs may only start once
                    # the previous pair's norm (att tile release) is emitted
                    while sum(len(v) for v in pv_pending.values()) > limit:
                        for o in sorted(pv_pending):
                            if o <= norm_cnt[0] and pv_pending[o]:
                                emit_pv(*pv_pending[o].pop(0))
                                if not pv_pending[o]:
                                    del pv_pending[o]
                                break
                        else:
                            return

                def emit_unit(jj, pair, kb):
                    ex = expp.tile([P, 1024], BF16, tag="ex",
                                   name=f"ex_{jj}_{pair}_{kb}")
                    ex_map[(jj, pair, kb)] = ex
                    sc = psp.tile([P, 1024], F32, tag="sc",
                                  name=f"sc_{jj}_{pair}_{kb}")
                    nc.tensor.matmul(
                        sc[:, 0:512],
                        lhsT=kt[0:D, kb * P:(kb + 1) * P],
                        rhs=qt[0:D, pair, jj * 512:(jj + 1) * 512],
                        start=True, stop=True)
                    nc.tensor.matmul(
                        sc[:, 512:1024],
                        lhsT=kt[D:P, kb * P:(kb + 1) * P],
                        rhs=qt[D:P, pair, jj * 512:(jj + 1) * 512],
                        start=True, stop=True)
                    nc.scalar.activation(
                        ex, sc, EXPF,
                        bias=bias_sb[:, kb:kb + 1], scale=1.0)
                    pv_pending.setdefault(2 * jj + pair, []).append(
                        (jj, pair, kb))
                    drain_pv(4)
                    if oproj_jobs:
                        if odelay[0] > 0:
                            odelay[0] -= 1
                        else:
                            emit_oproj_chunk(*oproj_jobs.pop(0))

                def emit_oproj_chunk(jj, c):
                    last = jj == QC - 1
                    slot = psp.tile([P, 1024], F32, tag="sc",
                                    name=f"o{jj}_{c}")
                    for t in range(2):
                        for n in range(2):
                            nc.tensor.matmul(
                                slot[:, n * 512:(n + 1) * 512],
                                lhsT=attnT_js[jj][:, t,
                                                  c * P:(c + 1) * P],
                                rhs=wo_sb[:, t, n * 512:(n + 1) * 512],
                                start=(t == 0), stop=(t == 1))
                    ot = osb.tile([P, 1024], BF16, tag="ot")
                    if last:
                        nc.scalar.copy(ot[:, 0:512], slot[:, 0:512])
                        nc.vector.tensor_copy(ot[:, 512:1024],
                                              slot[:, 512:1024])
                    else:
                        nc.vector.tensor_copy(ot, slot)
                    q0 = jj * 512 + c * P
                    for h in range(2):
                        q = queues[out_qi[0] % 2]  # sync/scalar
                        out_qi[0] += 1
                        q.dma_start(
                            out=out[q0:q0 + P, h * 512:(h + 1) * 512],
                            in_=ot[:, h * 512:(h + 1) * 512])

                # ---- schedule ----
                # proj0: k0 then q0 (paced by the chunked DMAs), copies
                # ordered so kt/qt land before the v path
                emit_kv(0, qfirst=True)
                # j0 wavefronted against proj1..proj3 (both pairs; p1 PVs
                # are order-gated and drain during later blocks)
                for t in range(1, QC):
                    kb0 = 4 * (t - 1)
                    emit_unit(0, 0, kb0)
                    emit_unit(0, 1, kb0)
                    emit_kv(t)
                    emit_unit(0, 0, kb0 + 1)
                    emit_unit(0, 1, kb0 + 1)
                    emit_vT(t)
                    emit_unit(0, 0, kb0 + 2)
                    emit_unit(0, 1, kb0 + 2)
                    emit_qproj(t)
                    emit_unit(0, 0, kb0 + 3)
                    emit_unit(0, 1, kb0 + 3)
                for kb in range(12, KB):
                    emit_unit(0, 0, kb)
                    emit_unit(0, 1, kb)
                # remaining blocks pair-major
                for jj in range(1, QC):
                    for pair in range(2):
                        for kb in range(KB):
                            emit_unit(jj, pair, kb)
                drain_pv(0)
                while oproj_jobs:
                    emit_oproj_chunk(*oproj_jobs.pop(0))

    nc.finalize()
    return nc


def make_in_maps(hidden_states, attention_mask, wq, wk, wv, wo):
    scale = D ** -0.5
    wq_s = (wq * scale).astype(np.float32)
    in_maps = []
    for c in range(NCORES):
        b = c // 4
        g = c % 4
        h0 = g * HEADS_PER_CORE * D  # first row of this core's q heads
        xTt = hidden_states[b].T  # [1024, 2048]
        # [p, j, kc, m] = xT[kc*128+p, j*512+m]
        xTc = np.ascontiguousarray(
            xTt.reshape(8, P, QC, 512).transpose(1, 2, 0, 3))
        wqTc = np.ascontiguousarray(wq_s[h0:h0 + 256, :].T)
        wkkTc = np.ascontiguousarray(
            np.concatenate([wk.T, wk.T], axis=1)).astype(np.float32)
        wvTc = np.ascontiguousarray(wv.T)
        woTc = np.ascontiguousarray(wo[:, h0:h0 + 256].T)
        bias = ((1.0 - attention_mask[b]) * -1e30 - LNSH).astype(np.float32)
        bias2d = np.ascontiguousarray(bias.reshape(KB, P).T)
        in_maps.append({
            "xT": xTc.astype(ml_dtypes.bfloat16),
            "wqT": wqTc.astype(ml_dtypes.bfloat16),
            "wkkT": wkkTc.astype(ml_dtypes.bfloat16),
            "wvT": wvTc.astype(ml_dtypes.bfloat16),
            "identT": np.eye(D, dtype=np.float32),
            "woT": woTc.astype(ml_dtypes.bfloat16),
            "bias2d": bias2d,
            "ones2d": np.ones((P, KB), dtype=ml_dtypes.bfloat16),
        })
    return in_maps


def run(inputs, trace=False, trace_cores=None):
    """Compile (cached) and run; returns (full_output, BassKernelResults)."""
    if "nc" not in _CACHE:
        _CACHE["nc"] = build_kernel()
    nc = _CACHE["nc"]
    in_maps = make_in_maps(**inputs)
    res = run_bass_kernel_spmd(
        nc, in_maps, list(range(NCORES)), trace=trace,
        trace_cores=trace_cores)
    parts = [res.results[c]["out"] for c in range(NCORES)]
    full = np.empty((B, S, HIDDEN), dtype=np.float32)
    for b in range(B):
        acc = np.zeros((S, HIDDEN), dtype=np.float64)
        for g in range(4):
            acc += parts[4 * b + g]
        full[b] = acc.astype(np.float32)
    return full, res


def kernel(hidden_states, attention_mask, wq, wk, wv, wo):
    full, _ = run(dict(hidden_states=np.asarray(hidden_states),
                       attention_mask=np.asarray(attention_mask),
                       wq=np.asarray(wq), wk=np.asarray(wk),
                       wv=np.asarray(wv), wo=np.asarray(wo)))
    return full


# revision 31
# speedup vs baseline: 1.0843x; 1.0843x over previous
"""MQA attention (16 Q heads, 1 KV head) on 8 trn2 NeuronCores.

Sharding: data-parallel on batch (2) x tensor-parallel on Q heads (4 per
core). Each core computes K/V for its batch (replicated within the batch
group), attention for its 4 heads, and a row-parallel o_proj partial; the
host sums the 4 partials per batch.

v3 kernel layout (all matmul contractions on partitions):
  xT [1024, 2048] (host pre-transposed); x/wq/wkk DMAs are kc-chunked
  across the 3 HWDGE queues so projection matmuls start ~4us in
  qT = wqT.T @ xT -> [256, 2048] as 2 head-pair tiles [128, 2048]
  kT duplicated to both partition halves -> row-packed score matmuls
  scoresT [k, q] per (j, pair, kb) in a shared PSUM ring [128, 1024]
  (the same ring carries proj and o_proj partials: 4 banks + 4 att banks)
  exp on ScalarE PSUM->SBUF bf16 with per-partition (=per-key) mask bias
  minus ln32 (cancels in softmax)
  PV: lhsT = vaug [128, 65] ([v | ones]) bf16, rhs = ex [128, 512] bf16,
  4-way interleaved accumulation (A/B heads x even/odd kb) to keep
  same-group matmuls apart; even+odd summed on DVE into the tmp tile
  normalization: reciprocal + DMA partition-broadcast + DVE multiply
  o_proj: out[q, hidden] partial = attnT.T @ woT, K=256, interleaved per-j

Schedule: scores/exp for (j0,p0) wavefront against the kc-paced DMAs and
the per-j projections (proj pieces scattered between units); all other
(j,pair) blocks run pair-major so each PV block gets a long reuse window.
ScalarE exp is the hard floor (~131us: 16.8M exps at 1 elem/cycle/lane);
the schedule keeps PE (the other ~150us floor) from ever blocking on it.
"""
import math
import sys

sys.path.insert(0, "/opt/trn_rl_repo")

import ml_dtypes
import numpy as np

import concourse.bass as bass
import concourse.bacc as bacc
import concourse.tile as tile
from concourse import mybir
from concourse.bass_utils import run_bass_kernel_spmd
from concourse.tile_rust import add_dep_helper

HIDDEN = 1024
NH = 16
D = 64
B = 2
S = 2048
NCORES = 8
HEADS_PER_CORE = 4
KB = S // 128   # 16 key blocks
QC = S // 512   # 4 query chunks
P = 128
LNSH = math.log(32.0)  # softmax shift; cancels in normalization

F32 = mybir.dt.float32
F32R = mybir.dt.float32r
BF16 = mybir.dt.bfloat16
EXPF = mybir.ActivationFunctionType.Exp
MULT = mybir.AluOpType.mult
ADD = mybir.AluOpType.add

_CACHE = {}


def build_kernel(num_devices=NCORES):
    nc = bacc.Bacc("TRN2", target_bir_lowering=False, debug=False,
                   num_devices=num_devices)

    xT = nc.dram_tensor("xT", [P, QC, 8, 512], BF16, kind="ExternalInput")
    wqT = nc.dram_tensor("wqT", [HIDDEN, 256], BF16, kind="ExternalInput")
    wkkT = nc.dram_tensor("wkkT", [HIDDEN, 128], BF16, kind="ExternalInput")
    wvT = nc.dram_tensor("wvT", [HIDDEN, D], BF16, kind="ExternalInput")
    identT = nc.dram_tensor("identT", [D, D], F32R, kind="ExternalInput")
    woT = nc.dram_tensor("woT", [256, HIDDEN], BF16, kind="ExternalInput")
    bias2d = nc.dram_tensor("bias2d", [P, KB], F32, kind="ExternalInput")
    ones2d = nc.dram_tensor("ones2d", [P, KB], BF16, kind="ExternalInput")
    out = nc.dram_tensor("out", [S, HIDDEN], BF16, kind="ExternalOutput")
    # internal DRAM bounce for the per-query 1/denom row broadcast
    bounce = nc.dram_tensor("bounce", [QC, 2, 2, 512], F32)

    queues = [nc.sync, nc.scalar, nc.gpsimd]

    with tile.TileContext(nc) as tc:
        with tc.tile_pool(name="persist", bufs=1) as persist:
            xts = [persist.tile([P, 8, 512], BF16, name=f"xt{jj}")
                   for jj in range(QC)]
            qt = persist.tile([P, 2, S], BF16)          # qT head pairs
            kt = persist.tile([P, S], BF16)             # kT dup both halves
            vt = persist.tile([D, S], F32R)
            vaug = persist.tile([P, KB, D + 1], BF16)   # [v | ones]
            attnT_js = [persist.tile([P, 2, 512], BF16, name=f"attnT{jj}")
                        for jj in range(QC)]
            wq_sb = persist.tile([P, 8, 256], BF16)
            wkk_sb = persist.tile([P, 8, 128], BF16)
            wv_sb = persist.tile([P, 8, D], BF16)
            id_sb = persist.tile([D, D], F32R)
            wo_sb = persist.tile([P, 2, HIDDEN], BF16)
            bias_sb = persist.tile([P, KB], F32)

            # ---- input DMAs ----
            # kc-chunked so k0/q0 matmuls can start as soon as the first
            # chunks land; queue kc%3 carries [wkk_kc, wq_kc, x0_kc].
            nc.sync.dma_start(out=bias_sb, in_=bias2d[:, :])
            warm = persist.tile([P, 1], F32)
            nc.scalar.activation(warm, bias_sb[:, 0:1], EXPF)

            wkk_ap = wkkT.ap().rearrange("(kc p) m -> p kc m", p=P)
            wq_ap = wqT.ap().rearrange("(kc p) m -> p kc m", p=P)
            nc.sync.dma_start(out=wkk_sb[:, 0:4, :], in_=wkk_ap[:, 0:4, :])
            nc.scalar.dma_start(out=wkk_sb[:, 4:8, :],
                                in_=wkk_ap[:, 4:8, :])
            nc.gpsimd.dma_start(out=wq_sb[:, 0:4, :], in_=wq_ap[:, 0:4, :])
            nc.scalar.dma_start(out=wq_sb[:, 4:8, :], in_=wq_ap[:, 4:8, :])
            for i, q in ((0, nc.sync), (1, nc.gpsimd),
                         (2, nc.scalar), (3, nc.sync)):
                q.dma_start(out=xts[0][:, 2 * i:2 * i + 2, :],
                            in_=xT[:, 0, 2 * i:2 * i + 2, :])
            nc.gpsimd.dma_start(
                out=wv_sb, in_=wvT.ap().rearrange("(kc p) m -> p kc m", p=P))
            nc.scalar.dma_start(out=id_sb, in_=identT[:, :])
            nc.scalar.dma_start(out=vaug[:, :, D:D + 1], in_=ones2d[:, :])
            for jj in range(1, QC):
                nc.sync.dma_start(out=xts[jj][:, 0:4, :],
                                  in_=xT[:, jj, 0:4, :])
                nc.scalar.dma_start(out=xts[jj][:, 4:8, :],
                                    in_=xT[:, jj, 4:8, :])
            for t in range(2):
                nc.gpsimd.dma_start(out=wo_sb[:, t, :],
                                    in_=woT[t * P:(t + 1) * P, :])

            # ---- PE warm-up burst: ~24 junk matmuls unthrottle the HAM
            # clock gate (4096-cycle activity window) before proj0 ----
            wjunk = persist.tile([P, 512], BF16)
            nc.vector.memset(wjunk, 0.0)
            with tc.tile_pool(name="warm_ps", bufs=1,
                              space="PSUM") as wps:
                wslot = wps.tile([P, 512], F32, tag="w")
                for _ in range(24):
                    nc.tensor.matmul(wslot, lhsT=wjunk[:, 0:128],
                                     rhs=wjunk, start=True, stop=True)

            with tc.tile_pool(name="ps", bufs=2, space="PSUM") as psp, \
                 tc.tile_pool(name="att_ps", bufs=1, space="PSUM") as attp, \
                 tc.tile_pool(name="ex_sb", bufs=8) as expp, \
                 tc.tile_pool(name="norm_sb", bufs=3) as normp, \
                 tc.tile_pool(name="o_sb", bufs=3) as osb:

                ex_map = {}
                att_map = {}
                pv_pending = {}   # pair ord -> [(jj, pair, kb), ...]
                normed = {jj: 0 for jj in range(QC)}
                norm_cnt = [0]
                out_qi = [0]
                oproj_jobs = []   # (jj, chunk) emitted spread-out, delayed
                odelay = [0]

                def emit_kv(jj, qfirst=False):
                    slot = psp.tile([P, 1024], F32, tag="sc", name=f"kv{jj}")
                    for kc in range(8):
                        nc.tensor.matmul(
                            slot[:, 0:512], lhsT=wkk_sb[:, kc, :],
                            rhs=xts[jj][:, kc, :],
                            start=(kc == 0), stop=(kc == 7))
                    nc.vector.tensor_copy(
                        kt[:, jj * 512:(jj + 1) * 512], slot[:, 0:512])
                    if qfirst:
                        emit_qproj(jj)
                    for kc in range(8):
                        nc.tensor.matmul(
                            slot[0:D, 512:1024], lhsT=wv_sb[:, kc, :],
                            rhs=xts[jj][:, kc, :],
                            start=(kc == 0), stop=(kc == 7))
                    nc.vector.tensor_copy(
                        vt[:, jj * 512:(jj + 1) * 512], slot[0:D, 512:1024])
                    if qfirst:
                        emit_vT(jj)

                def emit_vT(jj):
                    tslot = psp.tile([P, 1024], F32, tag="sc", name=f"vT{jj}")
                    tslot_r = tslot.bitcast(F32R)
                    for c in range(4):
                        blk = 4 * jj + c
                        nc.tensor.transpose(
                            tslot_r[:, c * D:(c + 1) * D],
                            vt[0:D, blk * P:(blk + 1) * P],
                            id_sb[:, :])
                    nc.vector.tensor_copy(
                        vaug[:, 4 * jj:4 * jj + 4, 0:D],
                        tslot_r[:, 0:4 * D].rearrange("p (c d) -> p c d", c=4))

                def emit_qproj(jj):
                    qslot = psp.tile([P, 1024], F32, tag="sc", name=f"q{jj}")
                    for pair in range(2):
                        for kc in range(8):
                            nc.tensor.matmul(
                                qslot[:, pair * 512:(pair + 1) * 512],
                                lhsT=wq_sb[:, kc, pair * P:(pair + 1) * P],
                                rhs=xts[jj][:, kc, :],
                                start=(kc == 0), stop=(kc == 7))
                    nc.vector.tensor_copy(
                        qt[:, :, jj * 512:(jj + 1) * 512],
                        qslot.rearrange("p (pr m) -> p pr m", pr=2))

                def emit_norm(jj, pair, tiles):
                    attAB_e, attAB_o = tiles
                    last = jj == QC - 1
                    dengs = {1: nc.sync if last else nc.gpsimd,
                             0: nc.gpsimd}
                    for h01 in (1, 0):  # head B first (its DMA is the tail)
                        hs = h01 * 512
                        deng = dengs[h01]
                        tmpe = normp.tile([D + 1, 512], F32, tag="tmpe")
                        nc.vector.tensor_copy(
                            tmpe, attAB_e[:, hs:hs + 512])
                        tmp = normp.tile([D + 1, 512], F32, tag="tmp")
                        nc.vector.scalar_tensor_tensor(
                            out=tmp, in0=attAB_o[:, hs:hs + 512],
                            scalar=1.0, in1=tmpe, op0=MULT, op1=ADD)
                        ds = normp.tile([D, 8], F32, tag="ds")
                        deng.dma_start(out=ds, in_=tmp[D:D + 1, :])
                        rs = normp.tile([D, 8], F32, tag="rs")
                        nc.vector.reciprocal(out=rs, in_=ds)
                        bc = normp.tile([D, 1, 512], F32, tag="bc")
                        wdma = deng.dma_start(
                            out=bounce[jj, pair, h01, :], in_=rs)
                        rdma = deng.dma_start(
                            out=bc,
                            in_=bounce[jj, pair,
                                       h01, :].partition_broadcast(D))
                        add_dep_helper(rdma.ins, wdma.ins,
                                       reason="bounce RAW")
                        if h01 == 0:
                            nc.vector.tensor_mul(
                                attnT_js[jj][0:D, pair, :],
                                tmp[0:D, :], bc[:, 0, :])
                        else:
                            nt = normp.tile([D, 512], BF16, tag="nt")
                            nc.vector.tensor_mul(nt, tmp[0:D, :],
                                                 bc[:, 0, :])
                            deng.dma_start(
                                out=attnT_js[jj][D:P, pair, :], in_=nt)
                    norm_cnt[0] += 1
                    normed[jj] += 1
                    if normed[jj] == 2:
                        oproj_jobs.extend((jj, c) for c in range(4))
                        odelay[0] = 10

                def emit_pv(jj, pair, kb):
                    if kb == 0:
                        tiles = tuple(
                            attp.tile([D + 1, 1024], F32, tag=tg,
                                      name=f"{tg}_{jj}_{pair}")
                            for tg in ("attE", "attO"))
                        att_map[(jj, pair)] = tiles
                    tiles = att_map[(jj, pair)]
                    eo = kb % 2
                    attP = tiles[eo]
                    ex = ex_map.pop((jj, pair, kb))
                    nc.tensor.matmul(
                        attP[:, 0:512], lhsT=vaug[:, kb, :],
                        rhs=ex[:, 0:512],
                        start=(kb == eo), stop=(kb == KB - 2 + eo))
                    nc.tensor.matmul(
                        attP[:, 512:1024], lhsT=vaug[:, kb, :],
                        rhs=ex[:, 512:1024],
                        start=(kb == eo), stop=(kb == KB - 2 + eo))
                    if kb == KB - 1:
                        emit_norm(jj, pair, tiles)
                        del att_map[(jj, pair)]

                def drain_pv(limit):
                    # pair-order gating: a pair's PVs may only start once
                    # the previous pair's norm (att tile release) is emitted
                    while sum(len(v) for v in pv_pending.values()) > limit:
                        for o in sorted(pv_pending):
                            if o <= norm_cnt[0] and pv_pending[o]:
                                emit_pv(*pv_pending[o].pop(0))
                                if not pv_pending[o]:
                                    del pv_pending[o]
                                break
                        else:
                            return

                def emit_unit(jj, pair, kb):
                    ex = expp.tile([P, 1024], BF16, tag="ex",
                                   name=f"ex_{jj}_{pair}_{kb}")
                    ex_map[(jj, pair, kb)] = ex
                    sc = psp.tile([P, 1024], F32, tag="sc",
                                  name=f"sc_{jj}_{pair}_{kb}")
                    nc.tensor.matmul(
                        sc[:, 0:512],
                        lhsT=kt[0:D, kb * P:(kb + 1) * P],
                        rhs=qt[0:D, pair, jj * 512:(jj + 1) * 512],
                        start=True, stop=True)
                    nc.tensor.matmul(
                        sc[:, 512:1024],
                        lhsT=kt[D:P, kb * P:(kb + 1) * P],
                        rhs=qt[D:P, pair, jj * 512:(jj + 1) * 512],
                        start=True, stop=True)
                    nc.scalar.activation(
                        ex, sc, EXPF,
                        bias=bias_sb[:, kb:kb + 1], scale=1.0)
                    pv_pending.setdefault(2 * jj + pair, []).append(
                        (jj, pair, kb))
                    drain_pv(4)
                    if oproj_jobs:
                        if odelay[0] > 0:
                            odelay[0] -= 1
                        else:
                            emit_oproj_chunk(*oproj_jobs.pop(0))

                def emit_oproj_chunk(jj, c):
                    last = jj == QC - 1
                    slot = psp.tile([P, 1024], F32, tag="sc",
                                    name=f"o{jj}_{c}")
                    for t in range(2):
                        for n in range(2):
                            nc.tensor.matmul(
                                slot[:, n * 512:(n + 1) * 512],
                                lhsT=attnT_js[jj][:, t,
                                                  c * P:(c + 1) * P],
                                rhs=wo_sb[:, t, n * 512:(n + 1) * 512],
                                start=(t == 0), stop=(t == 1))
                    ot = osb.tile([P, 1024], BF16, tag="ot")
                    if last:
                        nc.scalar.copy(ot[:, 0:512], slot[:, 0:512])
                        nc.vector.tensor_copy(ot[:, 512:1024],
                                              slot[:, 512:1024])
                    else:
                        nc.vector.tensor_copy(ot, slot)
                    q0 = jj * 512 + c * P
                    for h in range(2):
                        q = queues[out_qi[0] % 2]  # sync/scalar
                        out_qi[0] += 1
                        q.dma_start(
                            out=out[q0:q0 + P, h * 512:(h + 1) * 512],
                            in_=ot[:, h * 512:(h + 1) * 512])

                # ---- schedule ----
                # proj0: k0 then q0 (paced by the chunked DMAs), copies
                # ordered so kt/qt land before the v path
                emit_kv(0, qfirst=True)
                # j0 wavefronted against proj1..proj3 (both pairs; p1 PVs
                # are order-gated and drain during later blocks)
                for t in range(1, QC):
                    kb0 = 4 * (t - 1)
                    emit_unit(0, 0, kb0)
                    emit_kv(t)
                    emit_unit(0, 0, kb0 + 1)
                    emit_vT(t)
                    emit_unit(0, 0, kb0 + 2)
                    emit_qproj(t)
                    emit_unit(0, 0, kb0 + 3)
                for kb in range(12, KB):
                    emit_unit(0, 0, kb)
                # remaining blocks pair-major
                for jj in range(QC):
                    for pair in range(2):
                        if (jj, pair) == (0, 0):
                            continue
                        for kb in range(KB):
                            emit_unit(jj, pair, kb)
                drain_pv(0)
                while oproj_jobs:
                    emit_oproj_chunk(*oproj_jobs.pop(0))

    nc.finalize()
    return nc


def make_in_maps(hidden_states, attention_mask, wq, wk, wv, wo):
    scale = D ** -0.5
    wq_s = (wq * scale).astype(np.float32)
    in_maps = []
    for c in range(NCORES):
        b = c // 4
        g = c % 4
        h0 = g * HEADS_PER_CORE * D  # first row of this core's q heads
        xTt = hidden_states[b].T  # [1024, 2048]
        # [p, j, kc, m] = xT[kc*128+p, j*512+m]
        xTc = np.ascontiguousarray(
            xTt.reshape(8, P, QC, 512).transpose(1, 2, 0, 3))
        wqTc = np.ascontiguousarray(wq_s[h0:h0 + 256, :].T)
        wkkTc = np.ascontiguousarray(
            np.concatenate([wk.T, wk.T], axis=1)).astype(np.float32)
        wvTc = np.ascontiguousarray(wv.T)
        woTc = np.ascontiguousarray(wo[:, h0:h0 + 256].T)
        bias = ((1.0 - attention_mask[b]) * -1e30 - LNSH).astype(np.float32)
        bias2d = np.ascontiguousarray(bias.reshape(KB, P).T)
        in_maps.append({
            "xT": xTc.astype(ml_dtypes.bfloat16),
            "wqT": wqTc.astype(ml_dtypes.bfloat16),
            "wkkT": wkkTc.astype(ml_dtypes.bfloat16),
            "wvT": wvTc.astype(ml_dtypes.bfloat16),
            "identT": np.eye(D, dtype=np.float32),
            "woT": woTc.astype(ml_dtypes.bfloat16),
            "bias2d": bias2d,
            "ones2d": np.ones((P, KB), dtype=ml_dtypes.bfloat16),
        })
    return in_maps


def run(inputs, trace=False, trace_cores=None):
    """Compile (cached) and run; returns (full_output, BassKernelResults)."""
    if "nc" not in _CACHE:
        _CACHE["nc"] = build_kernel()
    nc = _CACHE["nc"]
    in_maps = make_in_maps(**inputs)
    res = run_bass_kernel_spmd(
        nc, in_maps, list(range(NCORES)), trace=trace,
        trace_cores=trace_cores)
    parts = [res.results[c]["out"] for c in range(NCORES)]
    full = np.empty((B, S, HIDDEN), dtype=np.float32)
    for b in range(B):
        acc = np.zeros((S, HIDDEN), dtype=np.float64)
        for g in range(4):
            acc += parts[4 * b + g]
        full[b] = acc.astype(np.float32)
    return full, res


def kernel(hidden_states, attention_mask, wq, wk, wv, wo):
    full, _ = run(dict(hidden_states=np.asarray(hidden_states),
                       attention_mask=np.asarray(attention_mask),
                       wq=np.asarray(wq), wk=np.asarray(wk),
                       wv=np.asarray(wv), wo=np.asarray(wo)))
    return full
